# revision 16
# baseline (speedup 1.0000x reference)
"""Trainium2 Bass kernel for nn_LocallyDense (grouped gather + per-group Dense
+ LeakyReLU + BatchNorm inference).

Sharding: expert-parallel over groups 0-39 (5 per core on 8 cores), plus a
K-split of group 40: each core computes a 192-row slice of its 1536-long
contraction (padded to 2 K-tiles with zeros) and stores the raw fp32
partial; the host sums the 8 partials and applies bias/LeakyReLU/BN for
that group. Every core runs the identical program shape.

The gather (x columns per group) and all BN constant math happen on the
HOST during input prep — the device program is a pure streamed GEMM in the
transposed formulation out^T[o, b] = W^T x^T:
  - lhsT (stationary) = W K-tile  [K=128, M=128 output-half]
  - rhs  (moving)     = gathered-x K-tile [K=128, N=256 batch]
  - PSUM accumulates 12 K-tiles per (group, output-half)
Gathered-x and W K-tiles are interleaved host-side into one combined DRAM
tensor; all loads stream on the Sync HWDGE queue (kept free of
compute-dependent work to avoid head-of-line blocking), all stores go on
the Scalar queue. Every chunk is SBUF-resident (no pool-reuse throttling)
so the loads run back-to-back at the HBM roofline. Epilogue is ACT Prelu
(bias as per-partition scalar AP — the transposed layout puts output
features on partitions) then one DVE tensor_scalar for the BN affine, with
inv = gamma/sqrt(var+eps), c = beta - mean*inv precomputed on host.
Outputs are stored bf16 and cast/transposed back on the host.
"""

import numpy as np
import ml_dtypes

B, D_IN, N_GROUPS, G, D_OUT = 256, 65536, 41, 1536, 256
BN_EPS = 1e-3
ALPHA = 0.3
N_CORES = 8
NGF = 5               # full groups per core
KT = G // 128         # 12 K-tiles per full group
MKT = 2               # K-tiles in the group-40 mini slice (192 rows + pad)
MROWS = G // N_CORES  # 192 real contraction rows per core for group 40
CW = B + D_OUT        # combined tile width per K-tile (x cols + w cols)
HC = KT // 2          # K-tiles per load chunk (half group)

USE_BF16 = True       # x/W feed the PE in bf16 (fp32 accumulate in PSUM)
TRACE = False         # set by test.py for profiling runs
TRACE_KW = {}
REPEAT = 1            # run the main loop R times (benchmarking differential)

_prog_cache = {}


def _np_dtx():
    return ml_dtypes.bfloat16 if USE_BF16 else np.float32


def _build_program(use_bf16: bool):
    import concourse.bacc as bacc
    import concourse.mybir as mybir
    import concourse.tile as tile

    f32 = mybir.dt.float32
    dt_x = mybir.dt.bfloat16 if use_bf16 else mybir.dt.float32

    nc = bacc.Bacc("TRN2", target_bir_lowering=False, debug=False,
                   num_devices=N_CORES)
    # layout: [mini (MKT k-tiles) | g0..g4 (KT k-tiles each)] x CW cols
    xw = nc.dram_tensor("xw", [128, (MKT + NGF * KT) * CW], dt_x,
                        kind="ExternalInput")
    # cols 0-9: bias[g, h*128+p] (g<5); 12-13: inv[h*128+p]; 14-15: c
    cons = nc.dram_tensor("cons", [128, 16], f32, kind="ExternalInput")
    out = nc.dram_tensor("out", [NGF * 2 * 128, B], dt_x,
                         kind="ExternalOutput")
    out40 = nc.dram_tensor("out40", [2 * 128, B], f32, kind="ExternalOutput")

    with tile.TileContext(nc) as tc:
        with tc.tile_pool(name="const", bufs=1) as cpool, \
             tc.tile_pool(name="xw", bufs=2 * NGF) as xwpool, \
             tc.tile_pool(name="mg", bufs=1) as mpool, \
             tc.tile_pool(name="ep", bufs=4) as epool, \
             tc.tile_pool(name="ps", bufs=3, space="PSUM") as ppool:

            # const tensor rides the store queue: the load queue must stay
            # dense so streaming starts at the first possible cycle
            ct = cpool.tile([128, 16], f32)
            nc.scalar.dma_start(out=ct[:], in_=cons[:, :])

            for rep in range(REPEAT):
                # Issue ALL loads up front (mini, then g0..g4), but emit the
                # PE work with g1 FIRST: the PE then blocks on g1's chunks
                # (4th/5th loads, ~4 transfers in), stays BEHIND the load
                # frontier the whole run, and never hits the
                # catch-up-stall → pstate-downshift cycle on the last group.
                # loads alternate across BOTH HWDGE queues (Sync/Scalar) so
                # per-dma_start descriptor-gen dead time overlaps transfers;
                # stores are emitted strictly AFTER all loads in each
                # queue's program order, so no load queues behind
                # compute-dependent work
                mt = mpool.tile([128, MKT, CW], dt_x, tag="mini",
                                name=f"mini_{rep}")
                nc.sync.dma_start(out=mt[:], in_=xw[:, 0:MKT * CW])
                chunks = {}
                for g in range(NGF):
                    for half in range(2):
                        xwt = xwpool.tile([128, HC, CW], dt_x, tag="xw")
                        base = (MKT + g * KT + half * HC) * CW
                        nc.sync.dma_start(
                            out=xwt[:], in_=xw[:, base:base + HC * CW])
                        chunks[(g, half)] = xwt

                def do_mini():
                    for h in range(2):
                        ps = ppool.tile([128, B], f32, tag=f"ps{h}",
                                        name=f"mps{h}_{rep}")
                        for kk in range(MKT):
                            nc.tensor.matmul(
                                out=ps[:],
                                lhsT=mt[:, kk, B + h * 128:B + (h + 1) * 128],
                                rhs=mt[:, kk, 0:B],
                                start=(kk == 0), stop=(kk == MKT - 1))
                        pt = epool.tile([128, B], f32, tag="t",
                                        name=f"mcp{h}_{rep}")
                        nc.vector.tensor_copy(pt[:], ps[:])
                        nc.scalar.dma_start(
                            out=out40[h * 128:(h + 1) * 128, :], in_=pt[:])

                pe_order = ["mini", 1, 0, 2, 3, 4]
                for g in pe_order:
                    if g == "mini":
                        do_mini()
                        continue
                    # h-halves interleaved per chunk: the second chunk's
                    # arrival only gates the final 12 matmuls of the group
                    pss = [ppool.tile([128, B], f32, tag=f"ps{h}",
                                      name=f"ps{h}_{rep}_{g}")
                           for h in range(2)]
                    for half in range(2):
                        xwt = chunks[(g, half)]
                        for h in range(2):
                            for c in range(HC):
                                nc.tensor.matmul(
                                    out=pss[h][:],
                                    lhsT=xwt[:, c,
                                             B + h * 128:B + (h + 1) * 128],
                                    rhs=xwt[:, c, 0:B],
                                    start=(half == 0 and c == 0),
                                    stop=(half == 1 and c == HC - 1))
                    for h in range(2):
                        t = epool.tile([128, B], f32, tag="t")
                        nc.scalar.activation(
                            out=t[:], in_=pss[h][:],
                            func=mybir.ActivationFunctionType.Prelu,
                            bias=ct[:, 2 * g + h:2 * g + h + 1],
                            scale=1.0, alpha=float(ALPHA))
                        y = epool.tile([128, B], dt_x, tag="y")
                        nc.vector.tensor_scalar(
                            out=y[:], in0=t[:],
                            scalar1=ct[:, 12 + h:13 + h],
                            scalar2=ct[:, 14 + h:15 + h],
                            op0=mybir.AluOpType.mult,
                            op1=mybir.AluOpType.add)
                        nc.scalar.dma_start(
                            out=out[(g * 2 + h) * 128:
                                    (g * 2 + h + 1) * 128, :],
                            in_=y[:])
    nc.compile()
    return nc


def _get_program(use_bf16: bool):
    key = (use_bf16, REPEAT)
    if key not in _prog_cache:
        _prog_cache[key] = _build_program(use_bf16)
    return _prog_cache[key]


def _prep_inputs(x, gidx, W, b, gamma, beta, mmean, mvar):
    dtx = _np_dtx()
    inv = (gamma.astype(np.float64) /
           np.sqrt(mvar.astype(np.float64) + BN_EPS)).astype(np.float32)
    cvec = (beta - mmean * inv).astype(np.float32)
    inv_pc = inv.reshape(2, 128).T      # [128, 2]
    c_pc = cvec.reshape(2, 128).T       # [128, 2]
    A40 = x[:, gidx[40]]                # [B, G] group-40 gather
    W40 = W[40]                         # [G, D_OUT]
    in_maps, metas = [], []
    for c in range(N_CORES):
        gs = list(range(5 * c, 5 * c + 5))
        gi = gidx[gs]                                    # [NGF, G]
        A = x[:, gi.reshape(-1)]                         # [B, NGF*G]
        xw = np.zeros((128, MKT + NGF * KT, CW), dtype=dtx)
        full = xw[:, MKT:, :].reshape(128, NGF, KT, CW)
        full[:, :, :, :B] = A.T.reshape(NGF, KT, 128, B) \
                             .transpose(2, 0, 1, 3)
        full[:, :, :, B:] = W[gs].reshape(NGF, KT, 128, D_OUT) \
                                 .transpose(2, 0, 1, 3)
        # group-40 K-slice: rows [MROWS*c, MROWS*(c+1)) padded to MKT tiles
        sl = slice(MROWS * c, MROWS * (c + 1))
        mx = np.zeros((MKT * 128, B), np.float32)
        mw = np.zeros((MKT * 128, D_OUT), np.float32)
        mx[:MROWS] = A40.T[sl]
        mw[:MROWS] = W40[sl]
        mini = xw[:, :MKT, :]
        mini[:, :, :B] = mx.reshape(MKT, 128, B).transpose(1, 0, 2)
        mini[:, :, B:] = mw.reshape(MKT, 128, D_OUT).transpose(1, 0, 2)
        cons = np.zeros((128, 16), np.float32)
        cons[:, 0:10] = b[gs].reshape(NGF, 2, 128).transpose(2, 0, 1) \
                             .reshape(128, 10)
        cons[:, 12:14] = inv_pc
        cons[:, 14:16] = c_pc
        in_maps.append({"xw": xw.reshape(128, (MKT + NGF * KT) * CW),
                        "cons": np.ascontiguousarray(cons)})
        metas.append(gs)
    return in_maps, metas


def kernel(**inputs):
    x = np.asarray(inputs["x"], dtype=np.float32)
    gidx = np.asarray(inputs["group_idx"]).astype(np.int64)
    W = np.asarray(inputs["W"], dtype=np.float32)
    b = np.asarray(inputs["b"], dtype=np.float32)
    gamma = np.asarray(inputs["gamma"], dtype=np.float32)
    beta = np.asarray(inputs["beta"], dtype=np.float32)
    mmean = np.asarray(inputs["moving_mean"], dtype=np.float32)
    mvar = np.asarray(inputs["moving_var"], dtype=np.float32)

    in_maps, metas = _prep_inputs(x, gidx, W, b, gamma, beta, mmean, mvar)
    nc = _get_program(USE_BF16)

    from concourse import bass_utils
    res = bass_utils.run_bass_kernel_spmd(
        nc, in_maps, core_ids=list(range(N_CORES)), trace=TRACE, **TRACE_KW)
    if TRACE:
        kernel.last_result = res

    full = np.empty((B, N_GROUPS, D_OUT), dtype=np.float32)
    z40 = np.zeros((D_OUT, B), np.float32)
    for c, gs in enumerate(metas):
        o = res.results[c]["out"].astype(np.float32) \
               .reshape(NGF, 2, 128, B)                   # [g, h, p, b]
        full[:, gs, :] = o.transpose(3, 0, 1, 2).reshape(B, NGF, D_OUT)
        z40 += res.results[c]["out40"]
    z = z40 + b[40][:, None]
    t = np.where(z >= 0, z, ALPHA * z)
    inv = (gamma.astype(np.float64) /
           np.sqrt(mvar.astype(np.float64) + BN_EPS)).astype(np.float32)
    cvec = (beta - mmean * inv).astype(np.float32)
    full[:, 40, :] = (t * inv[:, None] + cvec[:, None]).T
    return full


def host_check():
    """Validate host prep + unshard logic with a numpy matmul (no device)."""
    d = np.load("/root/problem/_ref_cache.npz")
    x = d["x"].astype(np.float32)
    gidx = d["group_idx"].astype(np.int64)
    W, b = d["W"].astype(np.float32), d["b"].astype(np.float32)
    gamma = d["gamma"].astype(np.float32)
    beta = d["beta"].astype(np.float32)
    mmean = d["moving_mean"].astype(np.float32)
    mvar = d["moving_var"].astype(np.float32)
    expected = d["expected"]
    in_maps, metas = _prep_inputs(x, gidx, W, b, gamma, beta, mmean, mvar)

    class FakeRes:
        pass

    res = FakeRes()
    res.results = []
    dtx = _np_dtx()
    for c in range(N_CORES):
        m = in_maps[c]
        xw = m["xw"].astype(np.float32).reshape(128, MKT + NGF * KT, CW)
        cons = m["cons"]
        o = np.empty((NGF, 2, 128, B), np.float32)
        for g in range(NGF):
            for h in range(2):
                ps = np.zeros((128, B), np.float32)
                for blk in range(KT):
                    tl = xw[:, MKT + g * KT + blk, :]
                    ps += tl[:, B + h * 128:B + (h + 1) * 128].T @ tl[:, 0:B]
                z = ps + cons[:, 2 * g + h:2 * g + h + 1]
                t = np.where(z >= 0, z, ALPHA * z)
                y = t * cons[:, 12 + h:13 + h] + cons[:, 14 + h:15 + h]
                o[g, h] = y.astype(dtx).astype(np.float32)
        p40 = np.zeros((2, 128, B), np.float32)
        for h in range(2):
            for kk in range(MKT):
                tl = xw[:, kk, :]
                p40[h] += tl[:, B + h * 128:B + (h + 1) * 128].T @ tl[:, 0:B]
        res.results.append({
            "out": o.reshape(NGF * 2 * 128, B).astype(dtx),
            "out40": p40.reshape(2 * 128, B)})

    # reuse kernel()'s unshard path
    full = np.empty((B, N_GROUPS, D_OUT), dtype=np.float32)
    z40 = np.zeros((D_OUT, B), np.float32)
    for c, gs in enumerate(metas):
        o = res.results[c]["out"].astype(np.float32).reshape(NGF, 2, 128, B)
        full[:, gs, :] = o.transpose(3, 0, 1, 2).reshape(B, NGF, D_OUT)
        z40 += res.results[c]["out40"]
    inv = (gamma.astype(np.float64) /
           np.sqrt(mvar.astype(np.float64) + BN_EPS)).astype(np.float32)
    cvec = (beta - mmean * inv).astype(np.float32)
    z = z40 + b[40][:, None]
    t = np.where(z >= 0, z, ALPHA * z)
    full[:, 40, :] = (t * inv[:, None] + cvec[:, None]).T
    err = np.max(np.abs(full - expected)) / (np.max(np.abs(expected)) + 1e-30)
    print(f"host_check max-abs-rel err = {err:.3e}")
    return err


if __name__ == "__main__":
    host_check()


# revision 17
# speedup vs baseline: 1.0233x; 1.0233x over previous
"""Trainium2 Bass kernel for nn_LocallyDense (grouped gather + per-group Dense
+ LeakyReLU + BatchNorm inference).

Sharding: expert-parallel over groups 0-39 (5 per core on 8 cores), plus a
K-split of group 40: each core computes a 192-row slice of its 1536-long
contraction (padded to 2 K-tiles with zeros) and stores the raw fp32
partial; the host sums the 8 partials and applies bias/LeakyReLU/BN for
that group. Every core runs the identical program shape.

The gather (x columns per group) and all BN constant math happen on the
HOST during input prep — the device program is a pure streamed GEMM in the
transposed formulation out^T[o, b] = W^T x^T:
  - lhsT (stationary) = W K-tile  [K=128, M=128 output-half]
  - rhs  (moving)     = gathered-x K-tile [K=128, N=256 batch]
  - PSUM accumulates 12 K-tiles per (group, output-half)
Gathered-x and W K-tiles are interleaved host-side into one combined DRAM
tensor; all loads stream on the Sync HWDGE queue (kept free of
compute-dependent work to avoid head-of-line blocking), all stores go on
the Scalar queue. Every chunk is SBUF-resident (no pool-reuse throttling)
so the loads run back-to-back at the HBM roofline. Epilogue is ACT Prelu
(bias as per-partition scalar AP — the transposed layout puts output
features on partitions) then one DVE tensor_scalar for the BN affine, with
inv = gamma/sqrt(var+eps), c = beta - mean*inv precomputed on host.
Outputs are stored bf16 and cast/transposed back on the host.
"""

import numpy as np
import ml_dtypes

B, D_IN, N_GROUPS, G, D_OUT = 256, 65536, 41, 1536, 256
BN_EPS = 1e-3
ALPHA = 0.3
N_CORES = 8
NGF = 5               # full groups per core
KT = G // 128         # 12 K-tiles per full group
MKT = 2               # K-tiles in the group-40 mini slice (192 rows + pad)
MROWS = G // N_CORES  # 192 real contraction rows per core for group 40
CW = B + D_OUT        # combined tile width per K-tile (x cols + w cols)
HC = KT // 2          # K-tiles per load chunk (half group)

USE_BF16 = True       # x/W feed the PE in bf16 (fp32 accumulate in PSUM)
TRACE = False         # set by test.py for profiling runs
TRACE_KW = {}
REPEAT = 1            # run the main loop R times (benchmarking differential)

_prog_cache = {}


def _np_dtx():
    return ml_dtypes.bfloat16 if USE_BF16 else np.float32


def _build_program(use_bf16: bool):
    import concourse.bacc as bacc
    import concourse.mybir as mybir
    import concourse.tile as tile

    f32 = mybir.dt.float32
    dt_x = mybir.dt.bfloat16 if use_bf16 else mybir.dt.float32

    nc = bacc.Bacc("TRN2", target_bir_lowering=False, debug=False,
                   num_devices=N_CORES)
    # layout: [mini (MKT k-tiles) | g0..g4 (KT k-tiles each)] x CW cols
    xw = nc.dram_tensor("xw", [128, (MKT + NGF * KT) * CW], dt_x,
                        kind="ExternalInput")
    # cols 0-9: bias[g, h*128+p] (g<5); 12-13: inv[h*128+p]; 14-15: c
    cons = nc.dram_tensor("cons", [128, 16], f32, kind="ExternalInput")
    out = nc.dram_tensor("out", [NGF * 2 * 128, B], dt_x,
                         kind="ExternalOutput")
    out40 = nc.dram_tensor("out40", [2 * 128, B], f32, kind="ExternalOutput")

    with tile.TileContext(nc) as tc:
        with tc.tile_pool(name="const", bufs=1) as cpool, \
             tc.tile_pool(name="xw", bufs=2 * NGF) as xwpool, \
             tc.tile_pool(name="mg", bufs=1) as mpool, \
             tc.tile_pool(name="ep", bufs=4) as epool, \
             tc.tile_pool(name="ps", bufs=3, space="PSUM") as ppool:

            # const tensor rides the store queue: the load queue must stay
            # dense so streaming starts at the first possible cycle
            ct = cpool.tile([128, 16], f32)
            nc.scalar.dma_start(out=ct[:], in_=cons[:, :])

            for rep in range(REPEAT):
                # Issue ALL loads up front (mini, then g0..g4) on the Sync
                # queue, but emit the PE work with g1 FIRST: the PE then
                # blocks on g1's chunks (4th/5th loads), stays BEHIND the
                # load frontier the whole run, and never hits the
                # catch-up-stall → pstate-downshift cycle on the last group.
                mt = mpool.tile([128, MKT, CW], dt_x, tag="mini",
                                name=f"mini_{rep}")
                nc.sync.dma_start(out=mt[:], in_=xw[:, 0:MKT * CW])
                chunks = {}
                for g in range(NGF):
                    for half in range(2):
                        xwt = xwpool.tile([128, HC, CW], dt_x, tag="xw")
                        base = (MKT + g * KT + half * HC) * CW
                        nc.sync.dma_start(
                            out=xwt[:], in_=xw[:, base:base + HC * CW])
                        chunks[(g, half)] = xwt

                def do_mini():
                    for h in range(2):
                        ps = ppool.tile([128, B], f32, tag=f"ps{h}",
                                        name=f"mps{h}_{rep}")
                        for kk in range(MKT):
                            nc.tensor.matmul(
                                out=ps[:],
                                lhsT=mt[:, kk, B + h * 128:B + (h + 1) * 128],
                                rhs=mt[:, kk, 0:B],
                                start=(kk == 0), stop=(kk == MKT - 1))
                        pt = epool.tile([128, B], f32, tag="t",
                                        name=f"mcp{h}_{rep}")
                        nc.vector.tensor_copy(pt[:], ps[:])
                        nc.scalar.dma_start(
                            out=out40[h * 128:(h + 1) * 128, :], in_=pt[:])

                pe_order = ["mini", 1, 0, 2, 3, 4]
                for g in pe_order:
                    if g == "mini":
                        do_mini()
                        continue
                    # h-halves interleaved per chunk: the second chunk's
                    # arrival only gates the final 12 matmuls of the group
                    pss = [ppool.tile([128, B], f32, tag=f"ps{h}",
                                      name=f"ps{h}_{rep}_{g}")
                           for h in range(2)]
                    for half in range(2):
                        xwt = chunks[(g, half)]
                        for h in range(2):
                            for c in range(HC):
                                nc.tensor.matmul(
                                    out=pss[h][:],
                                    lhsT=xwt[:, c,
                                             B + h * 128:B + (h + 1) * 128],
                                    rhs=xwt[:, c, 0:B],
                                    start=(half == 0 and c == 0),
                                    stop=(half == 1 and c == HC - 1))
                    for h in range(2):
                        t = epool.tile([128, B], f32, tag="t")
                        nc.scalar.activation(
                            out=t[:], in_=pss[h][:],
                            func=mybir.ActivationFunctionType.Prelu,
                            bias=ct[:, 2 * g + h:2 * g + h + 1],
                            scale=1.0, alpha=float(ALPHA))
                        y = epool.tile([128, B], dt_x, tag="y")
                        nc.vector.tensor_scalar(
                            out=y[:], in0=t[:],
                            scalar1=ct[:, 12 + h:13 + h],
                            scalar2=ct[:, 14 + h:15 + h],
                            op0=mybir.AluOpType.mult,
                            op1=mybir.AluOpType.add)
                        nc.scalar.dma_start(
                            out=out[(g * 2 + h) * 128:
                                    (g * 2 + h + 1) * 128, :],
                            in_=y[:])
    nc.compile()
    return nc


def _get_program(use_bf16: bool):
    key = (use_bf16, REPEAT)
    if key not in _prog_cache:
        _prog_cache[key] = _build_program(use_bf16)
    return _prog_cache[key]


def _prep_inputs(x, gidx, W, b, gamma, beta, mmean, mvar):
    dtx = _np_dtx()
    inv = (gamma.astype(np.float64) /
           np.sqrt(mvar.astype(np.float64) + BN_EPS)).astype(np.float32)
    cvec = (beta - mmean * inv).astype(np.float32)
    inv_pc = inv.reshape(2, 128).T      # [128, 2]
    c_pc = cvec.reshape(2, 128).T       # [128, 2]
    A40 = x[:, gidx[40]]                # [B, G] group-40 gather
    W40 = W[40]                         # [G, D_OUT]
    in_maps, metas = [], []
    for c in range(N_CORES):
        gs = list(range(5 * c, 5 * c + 5))
        gi = gidx[gs]                                    # [NGF, G]
        A = x[:, gi.reshape(-1)]                         # [B, NGF*G]
        xw = np.zeros((128, MKT + NGF * KT, CW), dtype=dtx)
        full = xw[:, MKT:, :].reshape(128, NGF, KT, CW)
        full[:, :, :, :B] = A.T.reshape(NGF, KT, 128, B) \
                             .transpose(2, 0, 1, 3)
        full[:, :, :, B:] = W[gs].reshape(NGF, KT, 128, D_OUT) \
                                 .transpose(2, 0, 1, 3)
        # group-40 K-slice: rows [MROWS*c, MROWS*(c+1)) padded to MKT tiles
        sl = slice(MROWS * c, MROWS * (c + 1))
        mx = np.zeros((MKT * 128, B), np.float32)
        mw = np.zeros((MKT * 128, D_OUT), np.float32)
        mx[:MROWS] = A40.T[sl]
        mw[:MROWS] = W40[sl]
        mini = xw[:, :MKT, :]
        mini[:, :, :B] = mx.reshape(MKT, 128, B).transpose(1, 0, 2)
        mini[:, :, B:] = mw.reshape(MKT, 128, D_OUT).transpose(1, 0, 2)
        cons = np.zeros((128, 16), np.float32)
        cons[:, 0:10] = b[gs].reshape(NGF, 2, 128).transpose(2, 0, 1) \
                             .reshape(128, 10)
        cons[:, 12:14] = inv_pc
        cons[:, 14:16] = c_pc
        in_maps.append({"xw": xw.reshape(128, (MKT + NGF * KT) * CW),
                        "cons": np.ascontiguousarray(cons)})
        metas.append(gs)
    return in_maps, metas


def kernel(**inputs):
    x = np.asarray(inputs["x"], dtype=np.float32)
    gidx = np.asarray(inputs["group_idx"]).astype(np.int64)
    W = np.asarray(inputs["W"], dtype=np.float32)
    b = np.asarray(inputs["b"], dtype=np.float32)
    gamma = np.asarray(inputs["gamma"], dtype=np.float32)
    beta = np.asarray(inputs["beta"], dtype=np.float32)
    mmean = np.asarray(inputs["moving_mean"], dtype=np.float32)
    mvar = np.asarray(inputs["moving_var"], dtype=np.float32)

    in_maps, metas = _prep_inputs(x, gidx, W, b, gamma, beta, mmean, mvar)
    nc = _get_program(USE_BF16)

    from concourse import bass_utils
    res = bass_utils.run_bass_kernel_spmd(
        nc, in_maps, core_ids=list(range(N_CORES)), trace=TRACE, **TRACE_KW)
    if TRACE:
        kernel.last_result = res

    full = np.empty((B, N_GROUPS, D_OUT), dtype=np.float32)
    z40 = np.zeros((D_OUT, B), np.float32)
    for c, gs in enumerate(metas):
        o = res.results[c]["out"].astype(np.float32) \
               .reshape(NGF, 2, 128, B)                   # [g, h, p, b]
        full[:, gs, :] = o.transpose(3, 0, 1, 2).reshape(B, NGF, D_OUT)
        z40 += res.results[c]["out40"]
    z = z40 + b[40][:, None]
    t = np.where(z >= 0, z, ALPHA * z)
    inv = (gamma.astype(np.float64) /
           np.sqrt(mvar.astype(np.float64) + BN_EPS)).astype(np.float32)
    cvec = (beta - mmean * inv).astype(np.float32)
    full[:, 40, :] = (t * inv[:, None] + cvec[:, None]).T
    return full


def host_check():
    """Validate host prep + unshard logic with a numpy matmul (no device)."""
    d = np.load("/root/problem/_ref_cache.npz")
    x = d["x"].astype(np.float32)
    gidx = d["group_idx"].astype(np.int64)
    W, b = d["W"].astype(np.float32), d["b"].astype(np.float32)
    gamma = d["gamma"].astype(np.float32)
    beta = d["beta"].astype(np.float32)
    mmean = d["moving_mean"].astype(np.float32)
    mvar = d["moving_var"].astype(np.float32)
    expected = d["expected"]
    in_maps, metas = _prep_inputs(x, gidx, W, b, gamma, beta, mmean, mvar)

    class FakeRes:
        pass

    res = FakeRes()
    res.results = []
    dtx = _np_dtx()
    for c in range(N_CORES):
        m = in_maps[c]
        xw = m["xw"].astype(np.float32).reshape(128, MKT + NGF * KT, CW)
        cons = m["cons"]
        o = np.empty((NGF, 2, 128, B), np.float32)
        for g in range(NGF):
            for h in range(2):
                ps = np.zeros((128, B), np.float32)
                for blk in range(KT):
                    tl = xw[:, MKT + g * KT + blk, :]
                    ps += tl[:, B + h * 128:B + (h + 1) * 128].T @ tl[:, 0:B]
                z = ps + cons[:, 2 * g + h:2 * g + h + 1]
                t = np.where(z >= 0, z, ALPHA * z)
                y = t * cons[:, 12 + h:13 + h] + cons[:, 14 + h:15 + h]
                o[g, h] = y.astype(dtx).astype(np.float32)
        p40 = np.zeros((2, 128, B), np.float32)
        for h in range(2):
            for kk in range(MKT):
                tl = xw[:, kk, :]
                p40[h] += tl[:, B + h * 128:B + (h + 1) * 128].T @ tl[:, 0:B]
        res.results.append({
            "out": o.reshape(NGF * 2 * 128, B).astype(dtx),
            "out40": p40.reshape(2 * 128, B)})

    # reuse kernel()'s unshard path
    full = np.empty((B, N_GROUPS, D_OUT), dtype=np.float32)
    z40 = np.zeros((D_OUT, B), np.float32)
    for c, gs in enumerate(metas):
        o = res.results[c]["out"].astype(np.float32).reshape(NGF, 2, 128, B)
        full[:, gs, :] = o.transpose(3, 0, 1, 2).reshape(B, NGF, D_OUT)
        z40 += res.results[c]["out40"]
    inv = (gamma.astype(np.float64) /
           np.sqrt(mvar.astype(np.float64) + BN_EPS)).astype(np.float32)
    cvec = (beta - mmean * inv).astype(np.float32)
    z = z40 + b[40][:, None]
    t = np.where(z >= 0, z, ALPHA * z)
    full[:, 40, :] = (t * inv[:, None] + cvec[:, None]).T
    err = np.max(np.abs(full - expected)) / (np.max(np.abs(expected)) + 1e-30)
    print(f"host_check max-abs-rel err = {err:.3e}")
    return err


if __name__ == "__main__":
    host_check()
